# revision 46
# baseline (speedup 1.0000x reference)
"""Differentiable top-k (Sinkhorn) Trainium2 kernel, v7.

Math: reference runs 100 log-domain Sinkhorn iterations on
log_P0[i,j] = -(s_i - sorted_j)^2/eps then sums exp(log_P) over the
first K=50 columns.  Relabeling rows by descending rank makes the
kernel matrix Kt[a,b] = exp(-(t_a-t_b)^2/eps) symmetric and the
alternating normalizations become one chain w_{k+1} = 1/(Kt w_k),
w_0 = 1.  For eps=1e-3 the chain converges so fast that N_STEPS=2
plain steps (no extrapolation) sit ~700x under the 2e-2 rel-err gate
(verified against the jax reference with fp16 Kt/iterates).  The
output P = diag(1/(Kt u)) Kt diag(u) is scale-invariant in u and in
any global scaling of Kt, needs u only on sorted blocks {0,1} and v
on block 0 (ranks >= 128 have exactly-zero top-50 mass, asserted
host-side).

v7 structure:
- all fp32 values reaching matmuls ride as exact fp16 h+l pairs
  (fp32 x = fp16(x) + fp16(x - fp16(x)) up to 2^-22; every
  comparison uses the same h+l proxy so the order is consistent,
  distinctness asserted host-side).  No fp32_mode matmuls at all.
- inputs are two 2KB f16 tensors per batch: the h/l rows (-> s_rep
  via one K=2 matmul against ones) and the h/l chunk columns in
  [2*NB,128] layout (-> PE-transposed into the sort weights; their
  sum is the rank-comparison scalar).  No big or strided DMAs.
- ranks: batch 0 counts s_i > s_j on DVE (CACHE_REDUCE accum), batch
  1 via ACT Sign with per-partition bias + accum (A = #lt - #gt).
- Kt via one ACT Derivative_Erf per block (erf'(d) = c exp(-d^2), c
  cancels); batch 0 distances on DVE, batch 1 on ACT Identity with
  bias = -RT*t_col.  A dummy erf' pins the single ACT table set.
- Kt tiles keep only (io,jo) pairs some chain/output matmul reads
  (with N_STEPS=2 nothing consumes block 3 rows), trimmed to the
  true band: |t_a - t_b| <= sqrt(16*eps), row starts 32-aligned
  (psum matmul base partition must be 0/32/64), row ends exact.
- output scatter produces a contiguous [1,512] row per batch -> one
  2KB DMA descriptor each.
"""

import numpy as np

import concourse.bacc as bacc
import concourse.mybir as mybir
from concourse import tile
from concourse.bass_utils import run_bass_kernel_spmd

F32 = mybir.dt.float32
F16 = mybir.dt.float16
F8 = mybir.dt.float8e4
BF16 = mybir.dt.bfloat16
I16 = mybir.dt.int16
I32 = mybir.dt.int32

B_FULL = 16
N = 512
NB = N // 128
TK = 50
EPS = 1e-3
N_STEPS = 2  # total chain steps (step 0 contracts w0 = ones)
N_CORES = 8
B_LOC = B_FULL // N_CORES
# beyond this distance exp(-d^2/eps) < 1.2e-7: numerically irrelevant
D_TRIM = float(np.sqrt(16.0 * EPS))
# fp32-exact-zero cutoff, used for the block-0 confinement assert
D_CUT = float(np.sqrt(87.5 * EPS))
RT = float(np.sqrt(1.0 / EPS))  # sqrt(1000)
WARM = 20
WARM2 = 34  # bridges the PE to the first sort matmul (idle downclocks)


def _used_pairs(blocks):
    """(io,jo) pairs actually contracted by the chain + output."""
    needed = [None] * N_STEPS
    needed[N_STEPS - 1] = [0, 1]
    for k in range(N_STEPS - 2, -1, -1):
        req = set()
        for io in needed[k + 1]:
            req.update(blocks[io])
        needed[k] = sorted(req)
    used = set()
    for k in range(N_STEPS):
        for io in needed[k]:
            for jo in blocks[io]:
                used.add((io, jo))
    for jo in blocks[0]:
        used.add((0, jo))  # pv and o50 read block-0 rows
    return needed, used


def _band_structure(scores):
    """Block band + per-(io,jo) trimmed row ranges of the sorted-score
    kernel matrix, unioned over all batches (SPMD: one program runs on
    every core)."""
    t = -np.sort(-scores.astype(np.float64), axis=-1)
    pairs = {}
    for b in range(scores.shape[0]):
        tb = t[b]
        for io in range(NB):
            ta = tb[io * 128 : (io + 1) * 128]
            for jo in range(NB):
                tj = tb[jo * 128 : (jo + 1) * 128]
                dmin = np.abs(ta[:, None] - tj[None, :]).min(axis=1)
                amask = dmin <= D_TRIM
                if not amask.any():
                    continue
                a_lo, a_hi = int(np.argmax(amask)), 128 - int(np.argmax(amask[::-1]))
                # psum matmul outputs must start at partition 0/32/64
                a_lo = min((a_lo // 32) * 32, 64)
                lo0, hi0 = pairs.get((io, jo), (128, 0))
                pairs[(io, jo)] = (min(lo0, a_lo), max(hi0, a_hi))
    blocks = {
        io: sorted(jo for (i, jo) in pairs if i == io) for io in range(NB)
    }
    for io in range(NB):
        assert io in blocks[io]
        assert pairs[(io, io)] == (0, 128)
    return blocks, pairs


def _check_block0_confined(scores):
    """Output (top-50 mass) must vanish for sorted ranks >= 128."""
    t = -np.sort(-scores.astype(np.float64), axis=-1)
    for b in range(scores.shape[0]):
        assert t[b, TK - 1] - t[b, 128] > D_CUT, "top-50 mass leaks past block 0"


def _build(blocks, pairs):
    nc = bacc.Bacc("TRN2", target_bir_lowering=False, debug=False)

    shlr_d = nc.declare_dram_parameter("s_hl_row", [B_LOC, 2, N], F16, isOutput=False)
    shlp_d = nc.declare_dram_parameter(
        "s_hl_parts", [B_LOC, 2 * NB, 128], F16, isOutput=False
    )
    out_d = nc.declare_dram_parameter("out", [B_LOC, N], F32, isOutput=True)

    needed, used = _used_pairs(blocks)
    io_span, tile_lo, tile_hi = {}, {}, {}
    for jo in range(NB):
        ios = sorted(io for (io, j) in used if j == jo)
        if not ios:
            continue
        assert ios == list(range(ios[0], ios[-1] + 1))
        io_span[jo] = ios
        tile_lo[jo] = ios[0] * 128 + pairs[(ios[0], jo)][0]
        tile_hi[jo] = ios[-1] * 128 + pairs[(ios[-1], jo)][1]

    AF = mybir.ActivationFunctionType
    OP = mybir.AluOpType

    with nc.allow_low_precision(reason="fp16 sinkhorn iterates"), \
         tile.TileContext(nc) as tc:
        with tc.tile_pool(name="sb", bufs=1) as sb, \
             tc.tile_pool(name="scr", bufs=2) as scr, \
             tc.tile_pool(name="wp", bufs=2) as wp, \
             tc.tile_pool(name="ps_rep", bufs=1, space="PSUM") as ps_rep, \
             tc.tile_pool(name="ps_tr", bufs=2, space="PSUM") as ps_tr, \
             tc.tile_pool(name="ps_sm", bufs=1, space="PSUM") as ps_sm, \
             tc.tile_pool(name="ps_out", bufs=1, space="PSUM") as ps_out:

            # ---- input DMAs: 4KB per batch, contiguous f16 ----
            shl_row, shl_p = {}, {}
            for b in range(B_LOC):
                shl_row[b] = sb.tile([2, N], F16, name=f"shlr{b}", tag=f"shlr{b}")
                shl_p[b] = sb.tile([2 * NB, 128], F16, name=f"shlp{b}", tag=f"shlp{b}")
            nc.sync.dma_start(shl_row[0][:], shlr_d[0])
            nc.sync.dma_start(shl_p[0][:], shlp_d[0])
            nc.scalar.dma_start(shl_row[1][:], shlr_d[1])
            nc.scalar.dma_start(shl_p[1][:], shlp_d[1])

            # ---- gpsimd: iotas ----
            iota_i = scr.tile([128, N], I16, name="iota_i", tag="iota_i")
            nc.gpsimd.iota(iota_i[:], pattern=[[1, N]], base=0,
                           channel_multiplier=0)
            iotac_i = scr.tile([128, 1], I32, name="iotac_i", tag="iotac_i")
            nc.gpsimd.iota(iotac_i[:], pattern=[[1, 1]], base=0,
                           channel_multiplier=1)

            # ---- DVE: consts, casts ----
            dummy16 = sb.tile([1, 128], F16, name="dummy16", tag="dummy16")
            nc.vector.memset(dummy16[:], 1.0)
            ones21 = sb.tile([2, 1], F16, name="ones21", tag="ones21")
            nc.vector.memset(ones21[:], 1.0)
            ones2r = sb.tile([2, 128], F16, name="ones2r", tag="ones2r")
            nc.vector.memset(ones2r[:], 1.0)
            w0ones = sb.tile([128, 1], F16, name="w0ones", tag="w0ones")
            nc.vector.memset(w0ones[:], 1.0)
            iota_rep = sb.tile([128, N], F16, name="iota_rep", tag="iota_rep")
            nc.vector.tensor_copy(iota_rep[:], iota_i[:])
            iota_col = sb.tile([128, 1], F32, name="iota_col", tag="iota_col")
            nc.vector.tensor_copy(iota_col[:], iotac_i[:])
            identity = sb.tile([128, 128], F16, name="identity", tag="identity")
            nc.vector.tensor_scalar(
                out=identity[:], in0=iota_rep[:, 0:128], scalar1=iota_col[:],
                scalar2=None, op0=OP.is_equal,
            )
            identity8 = sb.tile([128, 128], F8, name="identity8", tag="identity8")
            nc.vector.tensor_copy(identity8[:], identity[:])
            mask50 = sb.tile([128, 1], F16, name="mask50", tag="mask50")
            nc.vector.tensor_scalar(
                out=mask50[:], in0=iota_col[:], scalar1=float(TK),
                scalar2=None, op0=OP.is_lt,
            )

            # ---- ACT: force the erf_derivative table set to load now ----
            derf_warm = sb.tile([1, 128], F16, name="derf_warm", tag="derf_warm")
            nc.scalar.activation(derf_warm[:], dummy16[:], AF.Derivative_Erf)

            # ---- PE: warm-up; s_rep broadcasts; s_hl transposes ----
            warm_ps = ps_out.tile([128, 128], F32, name="warm", tag="or0")
            for _ in range(WARM):
                nc.tensor.matmul(
                    warm_ps[:], dummy16[:], dummy16[:], start=True, stop=True
                )
            s_rep_ps, shlt_ps = {}, {}
            for b in range(B_LOC):
                s_rep_ps[b] = ps_rep.tile([128, N], F32, name=f"srep{b}", tag=f"rep{b}")
                nc.tensor.matmul(
                    s_rep_ps[b][:], ones2r[:], shl_row[b][:], start=True, stop=True
                )
                # s_hlT[p, m] = shl_p[m, p]: the [128, 2NB] sort weights
                shlt_ps[b] = ps_tr.tile([128, 2 * NB], F16, name=f"shlt{b}", tag="tp")
                nc.tensor.transpose(
                    shlt_ps[b][:], shl_p[b][:], identity[0 : 2 * NB, 0 : 2 * NB]
                )
            for _ in range(WARM2):
                nc.tensor.matmul(
                    warm_ps[:], dummy16[:], dummy16[:], start=True, stop=True
                )

            s_hl, spa = {}, {}
            rankv, asum, pm = {}, {}, {}
            t_row_ps, t2h, t_rep_ps, tcol_ps, ntcol = {}, {}, {}, {}, {}
            kw, w16 = {}, {}
            tpose_ps, pmT = {}, {}
            u50, v0, os0 = {}, {}, {}

            def emit_shl(b):
                # spa = h+l is the rank scalar (must match s_rep's h+l sum
                # exactly; see module doc) - read straight from psum so the
                # rank ops don't wait for the sbuf weight copy
                s_hl[b] = sb.tile([128, 2 * NB], F16, name=f"s_hl{b}", tag=f"s_hl{b}")
                nc.vector.tensor_copy(s_hl[b][:], shlt_ps[b][:])
                spa[b] = sb.tile([128, NB], F32, name=f"spa{b}", tag=f"spa{b}")
                nc.vector.tensor_tensor(
                    out=spa[b][:], in0=s_hl[b][:, 0 : 2 * NB : 2],
                    in1=shlt_ps[b][:, 1 : 2 * NB : 2], op=OP.add,
                )

            def emit_ranks_dve(b):
                rankv[b] = sb.tile([128, NB], F32, name=f"rank{b}", tag=f"rank{b}")
                for c in range(NB):
                    junk = scr.tile([128, N], BF16, name=f"cmp{b}", tag=f"cmp{b}")
                    nc.vector.tensor_scalar(
                        out=junk[:],
                        in0=s_rep_ps[b][:],
                        scalar1=spa[b][:, c : c + 1],
                        scalar2=0.0,
                        op0=OP.is_gt,
                        op1=OP.add,
                        accum_out=rankv[b][:, c : c + 1],
                    )

            def emit_ranks_act(b):
                asum[b] = sb.tile([128, NB], F32, name=f"asum{b}", tag=f"asum{b}")
                for c in range(NB):
                    junk = scr.tile([128, N], BF16, name=f"sgn{b}", tag=f"sgn{b}")
                    nc.scalar.activation(
                        junk[:], s_rep_ps[b][:], AF.Sign,
                        bias=spa[b][:, c : c + 1], scale=-1.0,
                        accum_out=asum[b][:, c : c + 1],
                    )

            def emit_rank_transform(b):
                rankv[b] = sb.tile([128, NB], F32, name=f"rank{b}", tag=f"rank{b}")
                nc.vector.tensor_scalar(
                    out=rankv[b][:], in0=asum[b][:], scalar1=-0.5, scalar2=255.5,
                    op0=OP.mult, op1=OP.add,
                )

            def emit_pm(b):
                for c in range(NB):
                    pmt = sb.tile([128, N], F8, name=f"pm{b}_{c}", tag=f"pm{b}_{c}")
                    nc.vector.tensor_scalar(
                        out=pmt[:],
                        in0=iota_rep[:],
                        scalar1=rankv[b][:, c : c + 1],
                        scalar2=None,
                        op0=OP.is_equal,
                    )
                    pm[(b, c)] = pmt

            def emit_sort_mms(b):
                t_row_ps[b] = ps_sm.tile([2, N], F32, name=f"trps{b}", tag=f"sm{b}")
                for c in range(NB):
                    nc.tensor.matmul(
                        t_row_ps[b][:],
                        s_hl[b][:, 2 * c : 2 * c + 2],
                        pm[(b, c)][:],
                        start=(c == 0),
                        stop=(c == NB - 1),
                    )

            def emit_t2h(b):
                # exact f16 re-split of the sorted h/l rows
                t2h[b] = sb.tile([2, N], F16, name=f"t2h{b}", tag=f"t2h{b}")
                if b == 0:
                    nc.vector.tensor_copy(t2h[b][:], t_row_ps[b][:])
                else:
                    nc.scalar.copy(t2h[b][:], t_row_ps[b][:])

            def emit_tcol_trep_mms(b):
                tcol_ps[b] = ps_sm.tile([128, NB], F32, name=f"tcps{b}", tag=f"sm{b}")
                for c in range(NB):
                    nc.tensor.matmul(
                        tcol_ps[b][:, c : c + 1],
                        t2h[b][:, c * 128 : (c + 1) * 128],
                        ones21[:],
                        start=True,
                        stop=True,
                    )
                t_rep_ps[b] = ps_rep.tile([128, N], F32, name=f"trep{b}", tag=f"rep{b}")
                nc.tensor.matmul(
                    t_rep_ps[b][:], ones2r[:], t2h[b][:], start=True, stop=True
                )

            def emit_kt(b):
                for jo in sorted(io_span):
                    lo, hi = tile_lo[jo], tile_hi[jo]
                    dt = scr.tile([128, hi - lo], F16, name=f"d{b}", tag=f"d{b}")
                    nc.vector.tensor_scalar(
                        out=dt[:],
                        in0=t_rep_ps[b][:, lo:hi],
                        scalar1=tcol_ps[b][:, jo : jo + 1],
                        scalar2=RT,
                        op0=OP.subtract,
                        op1=OP.mult,
                    )
                    kt = sb.tile([128, hi - lo], F16, name=f"kt{b}_{jo}", tag=f"kt{b}_{jo}")
                    nc.scalar.activation(kt[:], dt[:], AF.Derivative_Erf)
                    kw[(b, jo)] = kt

            def _mm_io(b, pw, io, jo, rhs_col, start, stop):
                a_lo, a_hi = pairs[(io, jo)]
                c_lo = io * 128 + a_lo - tile_lo[jo]
                c_hi = io * 128 + a_hi - tile_lo[jo]
                nc.tensor.matmul(
                    pw[a_lo:a_hi, io : io + 1],
                    kw[(b, jo)][:, c_lo:c_hi],
                    rhs_col,
                    start=start,
                    stop=stop,
                )

            def emit_step(b, k):
                ios = needed[k]
                ncols = ios[-1] + 1
                pw = ps_sm.tile([128, NB], F32, name=f"pw{b}", tag=f"sm{b}")
                for io in ios:
                    jos = [io] + [j for j in blocks[io] if j != io]
                    for ji, jo in enumerate(jos):
                        rhs = w0ones[:] if k == 0 else w16[b][:, jo : jo + 1]
                        _mm_io(b, pw, io, jo, rhs, ji == 0, ji == len(jos) - 1)
                wn = wp.tile([128, NB], F16, name=f"w{b}", tag=f"w{b}")
                nc.vector.reciprocal(wn[:, 0:ncols], pw[:, 0:ncols])
                w16[b] = wn

            def emit_pm_transposes(b):
                for c in range(NB):
                    # fp8 transpose mode requires output element step 2
                    tp = ps_tr.tile([128, 256], F8, name=f"tp{b}_{c}", tag="tp")
                    nc.tensor.transpose(
                        tp[:, 0:256:2], pm[(b, c)][:, 0:128], identity8[:]
                    )
                    tpose_ps[(b, c)] = tp

            def emit_pmT_copies(b):
                for c in range(NB):
                    pt = sb.tile([128, 128], F16, name=f"pmT{b}_{c}", tag=f"pmT{b}_{c}")
                    nc.scalar.copy(pt[:], tpose_ps[(b, c)][:, 0:256:2])
                    pmT[(b, c)] = pt

            def emit_u50(b):
                u50[b] = sb.tile([128, 1], F16, name=f"u50{b}", tag=f"u50{b}")
                nc.gpsimd.tensor_tensor(
                    out=u50[b][:], in0=w16[b][:, 0:1], in1=mask50[:], op=OP.mult
                )

            def emit_pv(b):
                pv = ps_sm.tile([128, NB], F32, name=f"pv{b}", tag=f"sm{b}")
                jos0 = [0] + [j for j in blocks[0] if j != 0]
                for ji, jo in enumerate(jos0):
                    _mm_io(b, pv, 0, jo, w16[b][:, jo : jo + 1],
                           ji == 0, ji == len(jos0) - 1)
                nc.tensor.matmul(
                    pv[:, 1:2],
                    kw[(b, 0)][:, 0 - tile_lo[0] : 128 - tile_lo[0]],
                    u50[b][:],
                    start=True,
                    stop=True,
                )
                return pv

            def emit_os0(b, pv):
                v0[b] = sb.tile([128, 1], F32, name=f"v0{b}", tag=f"v0{b}")
                nc.vector.reciprocal(v0[b][:], pv[:, 0:1])
                os0[b] = sb.tile([128, 1], F16, name=f"os0{b}", tag=f"os0{b}")
                nc.vector.tensor_tensor(
                    out=os0[b][:], in0=v0[b][:], in1=pv[:, 1:2], op=OP.mult
                )

            def emit_scatter(b):
                orp = ps_out.tile([1, N], F32, name=f"or{b}", tag=f"or{b}")
                for c in range(NB):
                    nc.tensor.matmul(
                        orp[0:1, c * 128 : (c + 1) * 128],
                        os0[b][:],
                        pmT[(b, c)][:],
                        start=True,
                        stop=True,
                    )
                out_row = sb.tile([1, N], F32, name=f"orow{b}", tag=f"orow{b}")
                nc.vector.tensor_copy(out_row[:], orp[:])
                nc.sync.dma_start(
                    out_d[b].rearrange("(o n) -> o n", o=1), out_row[:]
                )

            # ---- emission schedule ----
            emit_shl(0)
            emit_shl(1)
            emit_ranks_act(1)   # ACT Sign b1
            emit_ranks_dve(0)   # DVE counts b0 concurrently
            emit_pm(0)
            emit_rank_transform(1)
            emit_pm(1)
            emit_sort_mms(0)
            emit_t2h(0)
            emit_pm_transposes(0)
            emit_pmT_copies(0)
            emit_tcol_trep_mms(0)
            emit_sort_mms(1)
            emit_t2h(1)
            emit_pm_transposes(1)
            emit_pmT_copies(1)
            emit_tcol_trep_mms(1)
            emit_kt(0)
            emit_kt(1)
            for k in range(N_STEPS):
                emit_step(0, k)
            emit_u50(0)
            pv0 = emit_pv(0)
            emit_os0(0, pv0)
            for k in range(N_STEPS):
                emit_step(1, k)
            emit_u50(1)
            emit_scatter(0)
            pv1 = emit_pv(1)
            emit_os0(1, pv1)
            emit_scatter(1)

    nc.compile()
    return nc


def kernel(scores):
    scores = np.ascontiguousarray(np.asarray(scores, dtype=np.float32))
    assert scores.shape == (B_FULL, N)
    h = scores.astype(np.float16)
    l = (scores - h.astype(np.float32)).astype(np.float16)
    approx = h.astype(np.float32) + l.astype(np.float32)
    for b in range(B_FULL):
        # the comparison-count sort assumes distinct scores per batch,
        # including after the exact-fp16-pair approximation (~2^-22 rel)
        assert np.unique(scores[b]).size == N, "tied scores unsupported"
        assert np.unique(approx[b]).size == N, "h+l approximation ties"
    blocks, pairs = _band_structure(scores)
    _check_block0_confined(scores)
    nc = _build(blocks, pairs)

    # h/l chunk columns in [2*NB, 128] layout: row 2c = h of chunk c
    hl_parts = np.empty((B_FULL, 2 * NB, 128), np.float16)
    for c in range(NB):
        hl_parts[:, 2 * c] = h[:, c * 128 : (c + 1) * 128]
        hl_parts[:, 2 * c + 1] = l[:, c * 128 : (c + 1) * 128]

    in_maps = []
    for cr in range(N_CORES):
        sl = slice(cr * B_LOC, (cr + 1) * B_LOC)
        in_maps.append({
            "s_hl_row": np.ascontiguousarray(np.stack([h[sl], l[sl]], axis=1)),
            "s_hl_parts": np.ascontiguousarray(hl_parts[sl]),
        })
    res = run_bass_kernel_spmd(nc, in_maps, core_ids=list(range(N_CORES)))
    return np.concatenate(
        [res.results[cr]["out"] for cr in range(N_CORES)], axis=0
    ).astype(np.float32)


# revision 48
# speedup vs baseline: 1.0473x; 1.0473x over previous
"""Differentiable top-k (Sinkhorn) Trainium2 kernel, v7.

Math: reference runs 100 log-domain Sinkhorn iterations on
log_P0[i,j] = -(s_i - sorted_j)^2/eps then sums exp(log_P) over the
first K=50 columns.  Relabeling rows by descending rank makes the
kernel matrix Kt[a,b] = exp(-(t_a-t_b)^2/eps) symmetric and the
alternating normalizations become one chain w_{k+1} = 1/(Kt w_k),
w_0 = 1.  For eps=1e-3 the chain converges so fast that N_STEPS=2
plain steps (no extrapolation) sit ~700x under the 2e-2 rel-err gate
(verified against the jax reference with fp16 Kt/iterates).  The
output P = diag(1/(Kt u)) Kt diag(u) is scale-invariant in u and in
any global scaling of Kt, needs u only on sorted blocks {0,1} and v
on block 0 (ranks >= 128 have exactly-zero top-50 mass, asserted
host-side).

v7 structure:
- all fp32 values reaching matmuls ride as exact fp16 h+l pairs
  (fp32 x = fp16(x) + fp16(x - fp16(x)) up to 2^-22; every
  comparison uses the same h+l proxy so the order is consistent,
  distinctness asserted host-side).  No fp32_mode matmuls at all.
- inputs are two 2KB f16 tensors per batch: the h/l rows (-> s_rep
  via one K=2 matmul against ones) and the h/l chunk columns in
  [2*NB,128] layout (-> PE-transposed into the sort weights; their
  sum is the rank-comparison scalar).  No big or strided DMAs.
- ranks: batch 0 counts s_i > s_j on DVE (CACHE_REDUCE accum), batch
  1 via ACT Sign with per-partition bias + accum (A = #lt - #gt).
- Kt via one ACT Derivative_Erf per block (erf'(d) = c exp(-d^2), c
  cancels); batch 0 distances on DVE, batch 1 on ACT Identity with
  bias = -RT*t_col.  A dummy erf' pins the single ACT table set.
- Kt tiles keep only (io,jo) pairs some chain/output matmul reads
  (with N_STEPS=2 nothing consumes block 3 rows), trimmed to the
  true band: |t_a - t_b| <= sqrt(16*eps), row starts 32-aligned
  (psum matmul base partition must be 0/32/64), row ends exact.
- output scatter produces a contiguous [1,512] row per batch -> one
  2KB DMA descriptor each.
"""

import numpy as np

import concourse.bacc as bacc
import concourse.mybir as mybir
from concourse import tile
from concourse.bass_utils import run_bass_kernel_spmd

F32 = mybir.dt.float32
F16 = mybir.dt.float16
F8 = mybir.dt.float8e4
BF16 = mybir.dt.bfloat16
I16 = mybir.dt.int16
I32 = mybir.dt.int32

B_FULL = 16
N = 512
NB = N // 128
TK = 50
EPS = 1e-3
N_STEPS = 1  # total chain steps (step 0 contracts w0 = ones; the
             # output's v = 1/(Kt u) acts as an implicit half-step, and
             # rows of P sum to 1 by construction - verified ~400x under
             # the rel-err gate vs the 100-iteration reference)
N_CORES = 8
B_LOC = B_FULL // N_CORES
# beyond this distance exp(-d^2/eps) < 1.2e-7: numerically irrelevant
D_TRIM = float(np.sqrt(16.0 * EPS))
# fp32-exact-zero cutoff, used for the block-0 confinement assert
D_CUT = float(np.sqrt(87.5 * EPS))
RT = float(np.sqrt(1.0 / EPS))  # sqrt(1000)
WARM = 20
WARM2 = 34  # bridges the PE to the first sort matmul (idle downclocks)


def _used_pairs(blocks):
    """(io,jo) pairs actually contracted by the chain + output."""
    needed = [None] * N_STEPS
    needed[N_STEPS - 1] = [0, 1]
    for k in range(N_STEPS - 2, -1, -1):
        req = set()
        for io in needed[k + 1]:
            req.update(blocks[io])
        needed[k] = sorted(req)
    used = set()
    for k in range(N_STEPS):
        for io in needed[k]:
            for jo in blocks[io]:
                used.add((io, jo))
    for jo in blocks[0]:
        used.add((0, jo))  # pv and o50 read block-0 rows
    return needed, used


def _band_structure(scores):
    """Block band + per-(io,jo) trimmed row ranges of the sorted-score
    kernel matrix, unioned over all batches (SPMD: one program runs on
    every core)."""
    t = -np.sort(-scores.astype(np.float64), axis=-1)
    pairs = {}
    for b in range(scores.shape[0]):
        tb = t[b]
        for io in range(NB):
            ta = tb[io * 128 : (io + 1) * 128]
            for jo in range(NB):
                tj = tb[jo * 128 : (jo + 1) * 128]
                dmin = np.abs(ta[:, None] - tj[None, :]).min(axis=1)
                amask = dmin <= D_TRIM
                if not amask.any():
                    continue
                a_lo, a_hi = int(np.argmax(amask)), 128 - int(np.argmax(amask[::-1]))
                # psum matmul outputs must start at partition 0/32/64
                a_lo = min((a_lo // 32) * 32, 64)
                lo0, hi0 = pairs.get((io, jo), (128, 0))
                pairs[(io, jo)] = (min(lo0, a_lo), max(hi0, a_hi))
    blocks = {
        io: sorted(jo for (i, jo) in pairs if i == io) for io in range(NB)
    }
    for io in range(NB):
        assert io in blocks[io]
        assert pairs[(io, io)] == (0, 128)
    return blocks, pairs


def _check_block0_confined(scores):
    """Output (top-50 mass) must vanish for sorted ranks >= 128."""
    t = -np.sort(-scores.astype(np.float64), axis=-1)
    for b in range(scores.shape[0]):
        assert t[b, TK - 1] - t[b, 128] > D_CUT, "top-50 mass leaks past block 0"


def _build(blocks, pairs):
    nc = bacc.Bacc("TRN2", target_bir_lowering=False, debug=False)

    shlr_d = nc.declare_dram_parameter("s_hl_row", [B_LOC, 2, N], F16, isOutput=False)
    shlp_d = nc.declare_dram_parameter(
        "s_hl_parts", [B_LOC, 2 * NB, 128], F16, isOutput=False
    )
    out_d = nc.declare_dram_parameter("out", [B_LOC, N], F32, isOutput=True)

    needed, used = _used_pairs(blocks)
    io_span, tile_lo, tile_hi = {}, {}, {}
    for jo in range(NB):
        ios = sorted(io for (io, j) in used if j == jo)
        if not ios:
            continue
        assert ios == list(range(ios[0], ios[-1] + 1))
        io_span[jo] = ios
        tile_lo[jo] = ios[0] * 128 + pairs[(ios[0], jo)][0]
        tile_hi[jo] = ios[-1] * 128 + pairs[(ios[-1], jo)][1]

    AF = mybir.ActivationFunctionType
    OP = mybir.AluOpType

    with nc.allow_low_precision(reason="fp16 sinkhorn iterates"), \
         tile.TileContext(nc) as tc:
        with tc.tile_pool(name="sb", bufs=1) as sb, \
             tc.tile_pool(name="scr", bufs=2) as scr, \
             tc.tile_pool(name="wp", bufs=2) as wp, \
             tc.tile_pool(name="ps_rep", bufs=1, space="PSUM") as ps_rep, \
             tc.tile_pool(name="ps_tr", bufs=2, space="PSUM") as ps_tr, \
             tc.tile_pool(name="ps_sm", bufs=1, space="PSUM") as ps_sm, \
             tc.tile_pool(name="ps_out", bufs=1, space="PSUM") as ps_out:

            # ---- input DMAs: 4KB per batch, contiguous f16 ----
            shl_row, shl_p = {}, {}
            for b in range(B_LOC):
                shl_row[b] = sb.tile([2, N], F16, name=f"shlr{b}", tag=f"shlr{b}")
                shl_p[b] = sb.tile([2 * NB, 128], F16, name=f"shlp{b}", tag=f"shlp{b}")
            nc.sync.dma_start(shl_row[0][:], shlr_d[0])
            nc.sync.dma_start(shl_p[0][:], shlp_d[0])
            nc.scalar.dma_start(shl_row[1][:], shlr_d[1])
            nc.scalar.dma_start(shl_p[1][:], shlp_d[1])

            # ---- gpsimd: iotas ----
            iota_i = scr.tile([128, N], I16, name="iota_i", tag="iota_i")
            nc.gpsimd.iota(iota_i[:], pattern=[[1, N]], base=0,
                           channel_multiplier=0)
            iotac_i = scr.tile([128, 1], I32, name="iotac_i", tag="iotac_i")
            nc.gpsimd.iota(iotac_i[:], pattern=[[1, 1]], base=0,
                           channel_multiplier=1)

            # ---- DVE: consts, casts ----
            dummy16 = sb.tile([1, 128], F16, name="dummy16", tag="dummy16")
            nc.vector.memset(dummy16[:], 1.0)
            ones21 = sb.tile([2, 1], F16, name="ones21", tag="ones21")
            nc.vector.memset(ones21[:], 1.0)
            ones2r = sb.tile([2, 128], F16, name="ones2r", tag="ones2r")
            nc.vector.memset(ones2r[:], 1.0)
            w0ones = sb.tile([128, 1], F16, name="w0ones", tag="w0ones")
            nc.vector.memset(w0ones[:], 1.0)
            iota_rep = sb.tile([128, N], F16, name="iota_rep", tag="iota_rep")
            nc.vector.tensor_copy(iota_rep[:], iota_i[:])
            iota_col = sb.tile([128, 1], F32, name="iota_col", tag="iota_col")
            nc.vector.tensor_copy(iota_col[:], iotac_i[:])
            identity = sb.tile([128, 128], F16, name="identity", tag="identity")
            nc.vector.tensor_scalar(
                out=identity[:], in0=iota_rep[:, 0:128], scalar1=iota_col[:],
                scalar2=None, op0=OP.is_equal,
            )
            identity8 = sb.tile([128, 128], F8, name="identity8", tag="identity8")
            nc.vector.tensor_copy(identity8[:], identity[:])
            mask50 = sb.tile([128, 1], F16, name="mask50", tag="mask50")
            nc.vector.tensor_scalar(
                out=mask50[:], in0=iota_col[:], scalar1=float(TK),
                scalar2=None, op0=OP.is_lt,
            )

            # ---- ACT: force the erf_derivative table set to load now ----
            derf_warm = sb.tile([1, 128], F16, name="derf_warm", tag="derf_warm")
            nc.scalar.activation(derf_warm[:], dummy16[:], AF.Derivative_Erf)

            # ---- PE: warm-up; s_rep broadcasts; s_hl transposes ----
            warm_ps = ps_out.tile([128, 128], F32, name="warm", tag="or0")
            for _ in range(WARM):
                nc.tensor.matmul(
                    warm_ps[:], dummy16[:], dummy16[:], start=True, stop=True
                )
            s_rep_ps, shlt_ps = {}, {}
            for b in range(B_LOC):
                s_rep_ps[b] = ps_rep.tile([128, N], F32, name=f"srep{b}", tag=f"rep{b}")
                nc.tensor.matmul(
                    s_rep_ps[b][:], ones2r[:], shl_row[b][:], start=True, stop=True
                )
                # s_hlT[p, m] = shl_p[m, p]: the [128, 2NB] sort weights
                shlt_ps[b] = ps_tr.tile([128, 2 * NB], F16, name=f"shlt{b}", tag="tp")
                nc.tensor.transpose(
                    shlt_ps[b][:], shl_p[b][:], identity[0 : 2 * NB, 0 : 2 * NB]
                )
            for _ in range(WARM2):
                nc.tensor.matmul(
                    warm_ps[:], dummy16[:], dummy16[:], start=True, stop=True
                )

            s_hl, spa = {}, {}
            rankv, asum, pm = {}, {}, {}
            t_row_ps, t2h, t_rep_ps, tcol_ps, ntcol = {}, {}, {}, {}, {}
            kw, w16 = {}, {}
            tpose_ps, pmT = {}, {}
            u50, v0, os0 = {}, {}, {}

            def emit_shl(b):
                # spa = h+l is the rank scalar (must match s_rep's h+l sum
                # exactly; see module doc) - read straight from psum so the
                # rank ops don't wait for the sbuf weight copy
                s_hl[b] = sb.tile([128, 2 * NB], F16, name=f"s_hl{b}", tag=f"s_hl{b}")
                nc.vector.tensor_copy(s_hl[b][:], shlt_ps[b][:])
                spa[b] = sb.tile([128, NB], F32, name=f"spa{b}", tag=f"spa{b}")
                nc.vector.tensor_tensor(
                    out=spa[b][:], in0=s_hl[b][:, 0 : 2 * NB : 2],
                    in1=shlt_ps[b][:, 1 : 2 * NB : 2], op=OP.add,
                )

            def emit_ranks_dve(b):
                rankv[b] = sb.tile([128, NB], F32, name=f"rank{b}", tag=f"rank{b}")
                for c in range(NB):
                    junk = scr.tile([128, N], BF16, name=f"cmp{b}", tag=f"cmp{b}")
                    nc.vector.tensor_scalar(
                        out=junk[:],
                        in0=s_rep_ps[b][:],
                        scalar1=spa[b][:, c : c + 1],
                        scalar2=0.0,
                        op0=OP.is_gt,
                        op1=OP.add,
                        accum_out=rankv[b][:, c : c + 1],
                    )

            def emit_ranks_act(b):
                asum[b] = sb.tile([128, NB], F32, name=f"asum{b}", tag=f"asum{b}")
                for c in range(NB):
                    junk = scr.tile([128, N], BF16, name=f"sgn{b}", tag=f"sgn{b}")
                    nc.scalar.activation(
                        junk[:], s_rep_ps[b][:], AF.Sign,
                        bias=spa[b][:, c : c + 1], scale=-1.0,
                        accum_out=asum[b][:, c : c + 1],
                    )

            def emit_rank_transform(b):
                rankv[b] = sb.tile([128, NB], F32, name=f"rank{b}", tag=f"rank{b}")
                nc.vector.tensor_scalar(
                    out=rankv[b][:], in0=asum[b][:], scalar1=-0.5, scalar2=255.5,
                    op0=OP.mult, op1=OP.add,
                )

            def emit_pm(b):
                for c in range(NB):
                    pmt = sb.tile([128, N], F8, name=f"pm{b}_{c}", tag=f"pm{b}_{c}")
                    nc.vector.tensor_scalar(
                        out=pmt[:],
                        in0=iota_rep[:],
                        scalar1=rankv[b][:, c : c + 1],
                        scalar2=None,
                        op0=OP.is_equal,
                    )
                    pm[(b, c)] = pmt

            def emit_sort_mms(b):
                t_row_ps[b] = ps_sm.tile([2, N], F32, name=f"trps{b}", tag=f"sm{b}")
                for c in range(NB):
                    nc.tensor.matmul(
                        t_row_ps[b][:],
                        s_hl[b][:, 2 * c : 2 * c + 2],
                        pm[(b, c)][:],
                        start=(c == 0),
                        stop=(c == NB - 1),
                    )

            def emit_t2h(b):
                # exact f16 re-split of the sorted h/l rows
                t2h[b] = sb.tile([2, N], F16, name=f"t2h{b}", tag=f"t2h{b}")
                if b == 0:
                    nc.vector.tensor_copy(t2h[b][:], t_row_ps[b][:])
                else:
                    nc.scalar.copy(t2h[b][:], t_row_ps[b][:])

            def emit_tcol_trep_mms(b):
                tcol_ps[b] = ps_sm.tile([128, NB], F32, name=f"tcps{b}", tag=f"sm{b}")
                for c in range(NB):
                    nc.tensor.matmul(
                        tcol_ps[b][:, c : c + 1],
                        t2h[b][:, c * 128 : (c + 1) * 128],
                        ones21[:],
                        start=True,
                        stop=True,
                    )
                t_rep_ps[b] = ps_rep.tile([128, N], F32, name=f"trep{b}", tag=f"rep{b}")
                nc.tensor.matmul(
                    t_rep_ps[b][:], ones2r[:], t2h[b][:], start=True, stop=True
                )

            def emit_kt(b):
                for jo in sorted(io_span):
                    lo, hi = tile_lo[jo], tile_hi[jo]
                    dt = scr.tile([128, hi - lo], F16, name=f"d{b}", tag=f"d{b}")
                    nc.vector.tensor_scalar(
                        out=dt[:],
                        in0=t_rep_ps[b][:, lo:hi],
                        scalar1=tcol_ps[b][:, jo : jo + 1],
                        scalar2=RT,
                        op0=OP.subtract,
                        op1=OP.mult,
                    )
                    kt = sb.tile([128, hi - lo], F16, name=f"kt{b}_{jo}", tag=f"kt{b}_{jo}")
                    nc.scalar.activation(kt[:], dt[:], AF.Derivative_Erf)
                    kw[(b, jo)] = kt

            def _mm_io(b, pw, io, jo, rhs_col, start, stop):
                a_lo, a_hi = pairs[(io, jo)]
                c_lo = io * 128 + a_lo - tile_lo[jo]
                c_hi = io * 128 + a_hi - tile_lo[jo]
                nc.tensor.matmul(
                    pw[a_lo:a_hi, io : io + 1],
                    kw[(b, jo)][:, c_lo:c_hi],
                    rhs_col,
                    start=start,
                    stop=stop,
                )

            def emit_step(b, k):
                ios = needed[k]
                ncols = ios[-1] + 1
                pw = ps_sm.tile([128, NB], F32, name=f"pw{b}", tag=f"sm{b}")
                for io in ios:
                    jos = [io] + [j for j in blocks[io] if j != io]
                    for ji, jo in enumerate(jos):
                        rhs = w0ones[:] if k == 0 else w16[b][:, jo : jo + 1]
                        _mm_io(b, pw, io, jo, rhs, ji == 0, ji == len(jos) - 1)
                wn = wp.tile([128, NB], F16, name=f"w{b}", tag=f"w{b}")
                nc.vector.reciprocal(wn[:, 0:ncols], pw[:, 0:ncols])
                w16[b] = wn

            def emit_pm_transposes(b):
                for c in range(NB):
                    # fp8 transpose mode requires output element step 2
                    tp = ps_tr.tile([128, 256], F8, name=f"tp{b}_{c}", tag="tp")
                    nc.tensor.transpose(
                        tp[:, 0:256:2], pm[(b, c)][:, 0:128], identity8[:]
                    )
                    tpose_ps[(b, c)] = tp

            def emit_pmT_copies(b):
                for c in range(NB):
                    pt = sb.tile([128, 128], F16, name=f"pmT{b}_{c}", tag=f"pmT{b}_{c}")
                    nc.scalar.copy(pt[:], tpose_ps[(b, c)][:, 0:256:2])
                    pmT[(b, c)] = pt

            def emit_u50(b):
                u50[b] = sb.tile([128, 1], F16, name=f"u50{b}", tag=f"u50{b}")
                nc.vector.tensor_tensor(
                    out=u50[b][:], in0=w16[b][:, 0:1], in1=mask50[:], op=OP.mult
                )

            def emit_pv(b):
                pv = ps_sm.tile([128, NB], F32, name=f"pv{b}", tag=f"sm{b}")
                jos0 = [0] + [j for j in blocks[0] if j != 0]
                for ji, jo in enumerate(jos0):
                    _mm_io(b, pv, 0, jo, w16[b][:, jo : jo + 1],
                           ji == 0, ji == len(jos0) - 1)
                nc.tensor.matmul(
                    pv[:, 1:2],
                    kw[(b, 0)][:, 0 - tile_lo[0] : 128 - tile_lo[0]],
                    u50[b][:],
                    start=True,
                    stop=True,
                )
                return pv

            def emit_os0(b, pv):
                v0[b] = sb.tile([128, 1], F32, name=f"v0{b}", tag=f"v0{b}")
                nc.vector.reciprocal(v0[b][:], pv[:, 0:1])
                os0[b] = sb.tile([128, 1], F16, name=f"os0{b}", tag=f"os0{b}")
                nc.vector.tensor_tensor(
                    out=os0[b][:], in0=v0[b][:], in1=pv[:, 1:2], op=OP.mult
                )

            def emit_scatter(b):
                orp = ps_out.tile([1, N], F32, name=f"or{b}", tag=f"or{b}")
                for c in range(NB):
                    nc.tensor.matmul(
                        orp[0:1, c * 128 : (c + 1) * 128],
                        os0[b][:],
                        pmT[(b, c)][:],
                        start=True,
                        stop=True,
                    )
                out_row = sb.tile([1, N], F32, name=f"orow{b}", tag=f"orow{b}")
                nc.vector.tensor_copy(out_row[:], orp[:])
                nc.sync.dma_start(
                    out_d[b].rearrange("(o n) -> o n", o=1), out_row[:]
                )

            # ---- emission schedule ----
            emit_shl(0)
            emit_shl(1)
            emit_ranks_act(1)   # ACT Sign b1
            emit_ranks_dve(0)   # DVE counts b0 concurrently
            emit_pm(0)
            emit_rank_transform(1)
            emit_pm(1)
            emit_sort_mms(0)
            emit_t2h(0)
            emit_pm_transposes(0)
            emit_pmT_copies(0)
            emit_tcol_trep_mms(0)
            emit_sort_mms(1)
            emit_t2h(1)
            emit_pm_transposes(1)
            emit_pmT_copies(1)
            emit_tcol_trep_mms(1)
            emit_kt(0)
            emit_kt(1)
            for k in range(N_STEPS):
                emit_step(0, k)
            emit_u50(0)
            pv0 = emit_pv(0)
            emit_os0(0, pv0)
            for k in range(N_STEPS):
                emit_step(1, k)
            emit_u50(1)
            emit_scatter(0)
            pv1 = emit_pv(1)
            emit_os0(1, pv1)
            emit_scatter(1)

    nc.compile()
    return nc


def kernel(scores):
    scores = np.ascontiguousarray(np.asarray(scores, dtype=np.float32))
    assert scores.shape == (B_FULL, N)
    h = scores.astype(np.float16)
    l = (scores - h.astype(np.float32)).astype(np.float16)
    approx = h.astype(np.float32) + l.astype(np.float32)
    for b in range(B_FULL):
        # the comparison-count sort assumes distinct scores per batch,
        # including after the exact-fp16-pair approximation (~2^-22 rel)
        assert np.unique(scores[b]).size == N, "tied scores unsupported"
        assert np.unique(approx[b]).size == N, "h+l approximation ties"
    blocks, pairs = _band_structure(scores)
    _check_block0_confined(scores)
    nc = _build(blocks, pairs)

    # h/l chunk columns in [2*NB, 128] layout: row 2c = h of chunk c
    hl_parts = np.empty((B_FULL, 2 * NB, 128), np.float16)
    for c in range(NB):
        hl_parts[:, 2 * c] = h[:, c * 128 : (c + 1) * 128]
        hl_parts[:, 2 * c + 1] = l[:, c * 128 : (c + 1) * 128]

    in_maps = []
    for cr in range(N_CORES):
        sl = slice(cr * B_LOC, (cr + 1) * B_LOC)
        in_maps.append({
            "s_hl_row": np.ascontiguousarray(np.stack([h[sl], l[sl]], axis=1)),
            "s_hl_parts": np.ascontiguousarray(hl_parts[sl]),
        })
    res = run_bass_kernel_spmd(nc, in_maps, core_ids=list(range(N_CORES)))
    return np.concatenate(
        [res.results[cr]["out"] for cr in range(N_CORES)], axis=0
    ).astype(np.float32)


# revision 49
# speedup vs baseline: 1.0596x; 1.0118x over previous
"""Differentiable top-k (Sinkhorn) Trainium2 kernel, v7.

Math: reference runs 100 log-domain Sinkhorn iterations on
log_P0[i,j] = -(s_i - sorted_j)^2/eps then sums exp(log_P) over the
first K=50 columns.  Relabeling rows by descending rank makes the
kernel matrix Kt[a,b] = exp(-(t_a-t_b)^2/eps) symmetric and the
alternating normalizations become one chain w_{k+1} = 1/(Kt w_k),
w_0 = 1.  For eps=1e-3 the chain converges so fast that N_STEPS=2
plain steps (no extrapolation) sit ~700x under the 2e-2 rel-err gate
(verified against the jax reference with fp16 Kt/iterates).  The
output P = diag(1/(Kt u)) Kt diag(u) is scale-invariant in u and in
any global scaling of Kt, needs u only on sorted blocks {0,1} and v
on block 0 (ranks >= 128 have exactly-zero top-50 mass, asserted
host-side).

v7 structure:
- all fp32 values reaching matmuls ride as exact fp16 h+l pairs
  (fp32 x = fp16(x) + fp16(x - fp16(x)) up to 2^-22; every
  comparison uses the same h+l proxy so the order is consistent,
  distinctness asserted host-side).  No fp32_mode matmuls at all.
- inputs are two 2KB f16 tensors per batch: the h/l rows (-> s_rep
  via one K=2 matmul against ones) and the h/l chunk columns in
  [2*NB,128] layout (-> PE-transposed into the sort weights; their
  sum is the rank-comparison scalar).  No big or strided DMAs.
- ranks: batch 0 counts s_i > s_j on DVE (CACHE_REDUCE accum), batch
  1 via ACT Sign with per-partition bias + accum (A = #lt - #gt).
- Kt via one ACT Derivative_Erf per block (erf'(d) = c exp(-d^2), c
  cancels); batch 0 distances on DVE, batch 1 on ACT Identity with
  bias = -RT*t_col.  A dummy erf' pins the single ACT table set.
- Kt tiles keep only (io,jo) pairs some chain/output matmul reads
  (with N_STEPS=2 nothing consumes block 3 rows), trimmed to the
  true band: |t_a - t_b| <= sqrt(16*eps), row starts 32-aligned
  (psum matmul base partition must be 0/32/64), row ends exact.
- output scatter produces a contiguous [1,512] row per batch -> one
  2KB DMA descriptor each.
"""

import numpy as np

import concourse.bacc as bacc
import concourse.mybir as mybir
from concourse import tile
from concourse.bass_utils import run_bass_kernel_spmd

F32 = mybir.dt.float32
F16 = mybir.dt.float16
F8 = mybir.dt.float8e4
BF16 = mybir.dt.bfloat16
I16 = mybir.dt.int16
I32 = mybir.dt.int32

B_FULL = 16
N = 512
NB = N // 128
TK = 50
EPS = 1e-3
N_STEPS = 1  # total chain steps (step 0 contracts w0 = ones; the
             # output's v = 1/(Kt u) acts as an implicit half-step, and
             # rows of P sum to 1 by construction - verified ~400x under
             # the rel-err gate vs the 100-iteration reference)
N_CORES = 8
B_LOC = B_FULL // N_CORES
# beyond this distance exp(-d^2/eps) < 1.2e-7: numerically irrelevant
D_TRIM = float(np.sqrt(16.0 * EPS))
# fp32-exact-zero cutoff, used for the block-0 confinement assert
D_CUT = float(np.sqrt(87.5 * EPS))
RT = float(np.sqrt(1.0 / EPS))  # sqrt(1000)
WARM = 20
WARM2 = 34  # bridges the PE to the first sort matmul (idle downclocks)


def _used_pairs(blocks):
    """(io,jo) pairs actually contracted by the chain + output."""
    needed = [None] * N_STEPS
    needed[N_STEPS - 1] = [0, 1]
    for k in range(N_STEPS - 2, -1, -1):
        req = set()
        for io in needed[k + 1]:
            req.update(blocks[io])
        needed[k] = sorted(req)
    used = set()
    for k in range(N_STEPS):
        for io in needed[k]:
            for jo in blocks[io]:
                used.add((io, jo))
    for jo in blocks[0]:
        used.add((0, jo))  # pv and o50 read block-0 rows
    return needed, used


def _band_structure(scores):
    """Block band + per-(io,jo) trimmed row ranges of the sorted-score
    kernel matrix, unioned over all batches (SPMD: one program runs on
    every core)."""
    t = -np.sort(-scores.astype(np.float64), axis=-1)
    pairs = {}
    for b in range(scores.shape[0]):
        tb = t[b]
        for io in range(NB):
            ta = tb[io * 128 : (io + 1) * 128]
            for jo in range(NB):
                tj = tb[jo * 128 : (jo + 1) * 128]
                dmin = np.abs(ta[:, None] - tj[None, :]).min(axis=1)
                amask = dmin <= D_TRIM
                if not amask.any():
                    continue
                a_lo, a_hi = int(np.argmax(amask)), 128 - int(np.argmax(amask[::-1]))
                # psum matmul outputs must start at partition 0/32/64
                a_lo = min((a_lo // 32) * 32, 64)
                lo0, hi0 = pairs.get((io, jo), (128, 0))
                pairs[(io, jo)] = (min(lo0, a_lo), max(hi0, a_hi))
    blocks = {
        io: sorted(jo for (i, jo) in pairs if i == io) for io in range(NB)
    }
    for io in range(NB):
        assert io in blocks[io]
        assert pairs[(io, io)] == (0, 128)
    return blocks, pairs


def _check_block0_confined(scores):
    """Output (top-50 mass) must vanish for sorted ranks >= 128."""
    t = -np.sort(-scores.astype(np.float64), axis=-1)
    for b in range(scores.shape[0]):
        assert t[b, TK - 1] - t[b, 128] > D_CUT, "top-50 mass leaks past block 0"


def _build(blocks, pairs):
    nc = bacc.Bacc("TRN2", target_bir_lowering=False, debug=False)

    shlr_d = nc.declare_dram_parameter("s_hl_row", [B_LOC, 2, N], F16, isOutput=False)
    shlp_d = nc.declare_dram_parameter(
        "s_hl_parts", [B_LOC, 2 * NB, 128], F16, isOutput=False
    )
    out_d = nc.declare_dram_parameter("out", [B_LOC, N], F32, isOutput=True)

    needed, used = _used_pairs(blocks)
    io_span, tile_lo, tile_hi = {}, {}, {}
    for jo in range(NB):
        ios = sorted(io for (io, j) in used if j == jo)
        if not ios:
            continue
        assert ios == list(range(ios[0], ios[-1] + 1))
        io_span[jo] = ios
        tile_lo[jo] = ios[0] * 128 + pairs[(ios[0], jo)][0]
        tile_hi[jo] = ios[-1] * 128 + pairs[(ios[-1], jo)][1]

    AF = mybir.ActivationFunctionType
    OP = mybir.AluOpType

    with nc.allow_low_precision(reason="fp16 sinkhorn iterates"), \
         tile.TileContext(nc) as tc:
        with tc.tile_pool(name="sb", bufs=1) as sb, \
             tc.tile_pool(name="scr", bufs=2) as scr, \
             tc.tile_pool(name="wp", bufs=2) as wp, \
             tc.tile_pool(name="ps_rep", bufs=1, space="PSUM") as ps_rep, \
             tc.tile_pool(name="ps_tr", bufs=2, space="PSUM") as ps_tr, \
             tc.tile_pool(name="ps_sm", bufs=1, space="PSUM") as ps_sm, \
             tc.tile_pool(name="ps_out", bufs=1, space="PSUM") as ps_out:

            # ---- input DMAs: 4KB per batch, contiguous f16 ----
            shl_row, shl_p = {}, {}
            for b in range(B_LOC):
                shl_row[b] = sb.tile([2, N], F16, name=f"shlr{b}", tag=f"shlr{b}")
                shl_p[b] = sb.tile([2 * NB, 128], F16, name=f"shlp{b}", tag=f"shlp{b}")
            nc.sync.dma_start(shl_row[0][:], shlr_d[0])
            nc.sync.dma_start(shl_p[0][:], shlp_d[0])
            nc.scalar.dma_start(shl_row[1][:], shlr_d[1])
            nc.scalar.dma_start(shl_p[1][:], shlp_d[1])

            # ---- gpsimd: iotas ----
            iota_i = scr.tile([128, N], I16, name="iota_i", tag="iota_i")
            nc.gpsimd.iota(iota_i[:], pattern=[[1, N]], base=0,
                           channel_multiplier=0)
            iotac_i = scr.tile([128, 1], I32, name="iotac_i", tag="iotac_i")
            nc.gpsimd.iota(iotac_i[:], pattern=[[1, 1]], base=0,
                           channel_multiplier=1)

            # ---- DVE: consts, casts ----
            dummy16 = sb.tile([1, 128], F16, name="dummy16", tag="dummy16")
            nc.vector.memset(dummy16[:], 1.0)
            ones21 = sb.tile([2, 1], F16, name="ones21", tag="ones21")
            nc.vector.memset(ones21[:], 1.0)
            ones2r = sb.tile([2, 128], F16, name="ones2r", tag="ones2r")
            nc.vector.memset(ones2r[:], 1.0)
            w0ones = sb.tile([128, 1], F16, name="w0ones", tag="w0ones")
            nc.vector.memset(w0ones[:], 1.0)
            iota_rep = sb.tile([128, N], F16, name="iota_rep", tag="iota_rep")
            nc.vector.tensor_copy(iota_rep[:], iota_i[:])
            iota_col = sb.tile([128, 1], F32, name="iota_col", tag="iota_col")
            nc.vector.tensor_copy(iota_col[:], iotac_i[:])
            identity = sb.tile([128, 128], F16, name="identity", tag="identity")
            nc.vector.tensor_scalar(
                out=identity[:], in0=iota_rep[:, 0:128], scalar1=iota_col[:],
                scalar2=None, op0=OP.is_equal,
            )
            identity8 = sb.tile([128, 128], F8, name="identity8", tag="identity8")
            nc.vector.tensor_copy(identity8[:], identity[:])
            mask50 = sb.tile([128, 1], F16, name="mask50", tag="mask50")
            nc.vector.tensor_scalar(
                out=mask50[:], in0=iota_col[:], scalar1=float(TK),
                scalar2=None, op0=OP.is_lt,
            )

            # ---- ACT: force the erf_derivative table set to load now ----
            derf_warm = sb.tile([1, 128], F16, name="derf_warm", tag="derf_warm")
            nc.scalar.activation(derf_warm[:], dummy16[:], AF.Derivative_Erf)

            # ---- PE: warm-up; s_rep broadcasts; s_hl transposes ----
            warm_ps = ps_out.tile([128, 128], F32, name="warm", tag="or0")
            for _ in range(WARM):
                nc.tensor.matmul(
                    warm_ps[:], dummy16[:], dummy16[:], start=True, stop=True
                )
            s_rep_ps, shlt_ps = {}, {}
            for b in range(B_LOC):
                s_rep_ps[b] = ps_rep.tile([128, N], F32, name=f"srep{b}", tag=f"rep{b}")
                nc.tensor.matmul(
                    s_rep_ps[b][:], ones2r[:], shl_row[b][:], start=True, stop=True
                )
                # s_hlT[p, m] = shl_p[m, p]: the [128, 2NB] sort weights
                shlt_ps[b] = ps_tr.tile([128, 2 * NB], F16, name=f"shlt{b}", tag="tp")
                nc.tensor.transpose(
                    shlt_ps[b][:], shl_p[b][:], identity[0 : 2 * NB, 0 : 2 * NB]
                )
            for _ in range(WARM2):
                nc.tensor.matmul(
                    warm_ps[:], dummy16[:], dummy16[:], start=True, stop=True
                )

            s_hl, spa = {}, {}
            rankv, asum, pm = {}, {}, {}
            t_row_ps, t2h, t_rep_ps, tcol_ps, ntcol = {}, {}, {}, {}, {}
            kw, w16 = {}, {}
            tpose_ps, pmT = {}, {}
            u50, v0, os0 = {}, {}, {}

            def emit_shl(b):
                # spa = h+l is the rank scalar (must match s_rep's h+l sum
                # exactly; see module doc) - read straight from psum so the
                # rank ops don't wait for the sbuf weight copy
                s_hl[b] = sb.tile([128, 2 * NB], F16, name=f"s_hl{b}", tag=f"s_hl{b}")
                nc.vector.tensor_copy(s_hl[b][:], shlt_ps[b][:])
                spa[b] = sb.tile([128, NB], F32, name=f"spa{b}", tag=f"spa{b}")
                nc.vector.tensor_tensor(
                    out=spa[b][:], in0=s_hl[b][:, 0 : 2 * NB : 2],
                    in1=shlt_ps[b][:, 1 : 2 * NB : 2], op=OP.add,
                )

            def emit_ranks_dve(b):
                rankv[b] = sb.tile([128, NB], F32, name=f"rank{b}", tag=f"rank{b}")
                for c in range(NB):
                    junk = scr.tile([128, N], BF16, name=f"cmp{b}", tag=f"cmp{b}")
                    nc.vector.tensor_scalar(
                        out=junk[:],
                        in0=s_rep_ps[b][:],
                        scalar1=spa[b][:, c : c + 1],
                        scalar2=0.0,
                        op0=OP.is_gt,
                        op1=OP.add,
                        accum_out=rankv[b][:, c : c + 1],
                    )

            def emit_ranks_act(b):
                asum[b] = sb.tile([128, NB], F32, name=f"asum{b}", tag=f"asum{b}")
                for c in range(NB):
                    junk = scr.tile([128, N], BF16, name=f"sgn{b}", tag=f"sgn{b}")
                    nc.scalar.activation(
                        junk[:], s_rep_ps[b][:], AF.Sign,
                        bias=spa[b][:, c : c + 1], scale=-1.0,
                        accum_out=asum[b][:, c : c + 1],
                    )

            def emit_rank_transform(b):
                rankv[b] = sb.tile([128, NB], F32, name=f"rank{b}", tag=f"rank{b}")
                nc.vector.tensor_scalar(
                    out=rankv[b][:], in0=asum[b][:], scalar1=-0.5, scalar2=255.5,
                    op0=OP.mult, op1=OP.add,
                )

            def emit_pm(b):
                for c in range(NB):
                    pmt = sb.tile([128, N], F16, name=f"pm{b}_{c}", tag=f"pm{b}_{c}")
                    nc.vector.tensor_scalar(
                        out=pmt[:],
                        in0=iota_rep[:],
                        scalar1=rankv[b][:, c : c + 1],
                        scalar2=None,
                        op0=OP.is_equal,
                    )
                    pm[(b, c)] = pmt

            def emit_sort_mms(b):
                t_row_ps[b] = ps_sm.tile([2, N], F32, name=f"trps{b}", tag=f"sm{b}")
                for c in range(NB):
                    nc.tensor.matmul(
                        t_row_ps[b][:],
                        s_hl[b][:, 2 * c : 2 * c + 2],
                        pm[(b, c)][:],
                        start=(c == 0),
                        stop=(c == NB - 1),
                    )

            def emit_t2h(b):
                # exact f16 re-split of the sorted h/l rows
                t2h[b] = sb.tile([2, N], F16, name=f"t2h{b}", tag=f"t2h{b}")
                if b == 0:
                    nc.vector.tensor_copy(t2h[b][:], t_row_ps[b][:])
                else:
                    nc.scalar.copy(t2h[b][:], t_row_ps[b][:])

            def emit_tcol_trep_mms(b):
                tcol_ps[b] = ps_sm.tile([128, NB], F32, name=f"tcps{b}", tag=f"sm{b}")
                for c in range(NB):
                    nc.tensor.matmul(
                        tcol_ps[b][:, c : c + 1],
                        t2h[b][:, c * 128 : (c + 1) * 128],
                        ones21[:],
                        start=True,
                        stop=True,
                    )
                t_rep_ps[b] = ps_rep.tile([128, N], F32, name=f"trep{b}", tag=f"rep{b}")
                nc.tensor.matmul(
                    t_rep_ps[b][:], ones2r[:], t2h[b][:], start=True, stop=True
                )

            def emit_kt(b):
                for jo in sorted(io_span):
                    lo, hi = tile_lo[jo], tile_hi[jo]
                    dt = scr.tile([128, hi - lo], F16, name=f"d{b}", tag=f"d{b}")
                    nc.vector.tensor_scalar(
                        out=dt[:],
                        in0=t_rep_ps[b][:, lo:hi],
                        scalar1=tcol_ps[b][:, jo : jo + 1],
                        scalar2=RT,
                        op0=OP.subtract,
                        op1=OP.mult,
                    )
                    kt = sb.tile([128, hi - lo], F16, name=f"kt{b}_{jo}", tag=f"kt{b}_{jo}")
                    nc.scalar.activation(kt[:], dt[:], AF.Derivative_Erf)
                    kw[(b, jo)] = kt

            def _mm_io(b, pw, io, jo, rhs_col, start, stop):
                a_lo, a_hi = pairs[(io, jo)]
                c_lo = io * 128 + a_lo - tile_lo[jo]
                c_hi = io * 128 + a_hi - tile_lo[jo]
                nc.tensor.matmul(
                    pw[a_lo:a_hi, io : io + 1],
                    kw[(b, jo)][:, c_lo:c_hi],
                    rhs_col,
                    start=start,
                    stop=stop,
                )

            def emit_step(b, k):
                ios = needed[k]
                ncols = ios[-1] + 1
                pw = ps_sm.tile([128, NB], F32, name=f"pw{b}", tag=f"sm{b}")
                for io in ios:
                    jos = [io] + [j for j in blocks[io] if j != io]
                    for ji, jo in enumerate(jos):
                        rhs = w0ones[:] if k == 0 else w16[b][:, jo : jo + 1]
                        _mm_io(b, pw, io, jo, rhs, ji == 0, ji == len(jos) - 1)
                wn = wp.tile([128, NB], F16, name=f"w{b}", tag=f"w{b}")
                nc.vector.reciprocal(wn[:, 0:ncols], pw[:, 0:ncols])
                w16[b] = wn

            def emit_pm_transposes(b):
                for c in range(NB):
                    tp = ps_tr.tile([128, 128], F16, name=f"tp{b}_{c}", tag="tp")
                    nc.tensor.transpose(tp[:], pm[(b, c)][:, 0:128], identity[:])
                    tpose_ps[(b, c)] = tp

            def emit_pmT_copies(b):
                for c in range(NB):
                    pt = sb.tile([128, 128], F16, name=f"pmT{b}_{c}", tag=f"pmT{b}_{c}")
                    nc.scalar.copy(pt[:], tpose_ps[(b, c)][:])
                    pmT[(b, c)] = pt

            def emit_u50(b):
                u50[b] = sb.tile([128, 1], F16, name=f"u50{b}", tag=f"u50{b}")
                nc.vector.tensor_tensor(
                    out=u50[b][:], in0=w16[b][:, 0:1], in1=mask50[:], op=OP.mult
                )

            def emit_pv(b):
                pv = ps_sm.tile([128, NB], F32, name=f"pv{b}", tag=f"sm{b}")
                jos0 = [0] + [j for j in blocks[0] if j != 0]
                for ji, jo in enumerate(jos0):
                    _mm_io(b, pv, 0, jo, w16[b][:, jo : jo + 1],
                           ji == 0, ji == len(jos0) - 1)
                nc.tensor.matmul(
                    pv[:, 1:2],
                    kw[(b, 0)][:, 0 - tile_lo[0] : 128 - tile_lo[0]],
                    u50[b][:],
                    start=True,
                    stop=True,
                )
                return pv

            def emit_os0(b, pv):
                v0[b] = sb.tile([128, 1], F32, name=f"v0{b}", tag=f"v0{b}")
                nc.vector.reciprocal(v0[b][:], pv[:, 0:1])
                os0[b] = sb.tile([128, 1], F16, name=f"os0{b}", tag=f"os0{b}")
                nc.vector.tensor_tensor(
                    out=os0[b][:], in0=v0[b][:], in1=pv[:, 1:2], op=OP.mult
                )

            def emit_scatter(b):
                orp = ps_out.tile([1, N], F32, name=f"or{b}", tag=f"or{b}")
                for c in range(NB):
                    nc.tensor.matmul(
                        orp[0:1, c * 128 : (c + 1) * 128],
                        os0[b][:],
                        pmT[(b, c)][:],
                        start=True,
                        stop=True,
                    )
                out_row = sb.tile([1, N], F32, name=f"orow{b}", tag=f"orow{b}")
                nc.vector.tensor_copy(out_row[:], orp[:])
                nc.sync.dma_start(
                    out_d[b].rearrange("(o n) -> o n", o=1), out_row[:]
                )

            # ---- emission schedule ----
            emit_shl(0)
            emit_shl(1)
            emit_ranks_act(1)   # ACT Sign b1
            emit_ranks_dve(0)   # DVE counts b0 concurrently
            emit_pm(0)
            emit_rank_transform(1)
            emit_pm(1)
            emit_sort_mms(0)
            emit_t2h(0)
            emit_pm_transposes(0)
            emit_pmT_copies(0)
            emit_tcol_trep_mms(0)
            emit_sort_mms(1)
            emit_t2h(1)
            emit_pm_transposes(1)
            emit_pmT_copies(1)
            emit_tcol_trep_mms(1)
            emit_kt(0)
            emit_kt(1)
            for k in range(N_STEPS):
                emit_step(0, k)
            emit_u50(0)
            pv0 = emit_pv(0)
            emit_os0(0, pv0)
            for k in range(N_STEPS):
                emit_step(1, k)
            emit_u50(1)
            pv1 = emit_pv(1)
            emit_os0(1, pv1)
            emit_scatter(1)
            emit_scatter(0)

    nc.compile()
    return nc


def kernel(scores):
    scores = np.ascontiguousarray(np.asarray(scores, dtype=np.float32))
    assert scores.shape == (B_FULL, N)
    h = scores.astype(np.float16)
    l = (scores - h.astype(np.float32)).astype(np.float16)
    approx = h.astype(np.float32) + l.astype(np.float32)
    for b in range(B_FULL):
        # the comparison-count sort assumes distinct scores per batch,
        # including after the exact-fp16-pair approximation (~2^-22 rel)
        assert np.unique(scores[b]).size == N, "tied scores unsupported"
        assert np.unique(approx[b]).size == N, "h+l approximation ties"
    blocks, pairs = _band_structure(scores)
    _check_block0_confined(scores)
    nc = _build(blocks, pairs)

    # h/l chunk columns in [2*NB, 128] layout: row 2c = h of chunk c
    hl_parts = np.empty((B_FULL, 2 * NB, 128), np.float16)
    for c in range(NB):
        hl_parts[:, 2 * c] = h[:, c * 128 : (c + 1) * 128]
        hl_parts[:, 2 * c + 1] = l[:, c * 128 : (c + 1) * 128]

    in_maps = []
    for cr in range(N_CORES):
        sl = slice(cr * B_LOC, (cr + 1) * B_LOC)
        in_maps.append({
            "s_hl_row": np.ascontiguousarray(np.stack([h[sl], l[sl]], axis=1)),
            "s_hl_parts": np.ascontiguousarray(hl_parts[sl]),
        })
    res = run_bass_kernel_spmd(nc, in_maps, core_ids=list(range(N_CORES)))
    return np.concatenate(
        [res.results[cr]["out"] for cr in range(N_CORES)], axis=0
    ).astype(np.float32)


# revision 50
# speedup vs baseline: 1.0652x; 1.0053x over previous
"""Differentiable top-k (Sinkhorn) Trainium2 kernel, v9.

Math: reference runs 100 log-domain Sinkhorn iterations on
log_P0[i,j] = -(s_i - sorted_j)^2/eps then sums exp(log_P) over the
first K=50 columns.  Relabeling rows by descending rank makes the
kernel matrix Kt[a,b] = exp(-(t_a-t_b)^2/eps) symmetric and the
alternating normalizations become one chain w_{k+1} = 1/(Kt w_k),
w_0 = 1.  For eps=1e-3 the chain converges so fast that a single
step u = 1/(Kt 1) (the output's v = 1/(Kt u) is an implicit second
half-step and rows of P sum to 1 by construction) lands ~370x under
the 2e-2 rel-err gate vs the 100-iteration reference (simulated in
fp64/fp16; hardware tracks the simulation within ~1.2x).  The output
P = diag(1/(Kt u)) Kt diag(u) is scale-invariant in u and in any
global scaling of Kt, needs u only on sorted blocks {0,1} and v on
block 0 (ranks >= 128 have exactly-zero top-50 mass, asserted
host-side).

Structure:
- all fp32 values reaching matmuls ride as exact fp16 h+l pairs
  (fp32 x ~= fp16(x) + fp16(x - fp16(x)) up to 2^-22; every
  comparison uses the same h+l proxy so the order is consistent,
  distinctness asserted host-side).  No fp32_mode=LOW_HIGH matmuls
  (those run every fp32 matmul twice and cost ~2.2us per broadcast).
- inputs are two 2KB f16 tensors per batch: the h/l rows (-> s_rep
  via one K=2 matmul against ones) and the h/l chunk columns in
  [2*NB,128] layout (-> PE-transposed into the M=2 sort weights;
  their sum is the rank-comparison scalar).  No big or strided DMAs
  (v2 shipped 645KB and strided stores burst into 512 4-byte packets
  whose completion trailed the kernel by ~8.5us).
- ranks: batch 0 counts s_i > s_j on DVE (CACHE_REDUCE accum), batch
  1 via ACT Sign with per-partition bias + accum (A = #lt - #gt), so
  both batches' rank phases run concurrently.  GPSIMD only gets
  iotas and SBUF-only ops: its AP-scalar tensor ops and PSUM access
  are rejected by codegen, and its ISA ops (partition_broadcast)
  stall ~10us in library loads.
- Kt via one DVE distance op + one ACT Derivative_Erf per block:
  erf'(d) = (2/sqrt(pi)) exp(-d^2) and the constant cancels by scale
  invariance, so no Square/Exp pair.  A dummy erf' pins the single
  ACT table set (erf_derivative also holds Sign/Copy/Identity).
- Kt tiles keep only the (io,jo) block pairs some chain/output
  matmul reads (with one chain step nothing consumes blocks 2-3 rows
  except block 1's row sums), trimmed to the true band
  (|t_a - t_b| <= sqrt(16*eps); beyond it entries are < 1.2e-7 and
  verified irrelevant).  Chain matmul rows start 32-aligned (psum
  matmul base partition must be 0/32/64), row ends are exact.
- output scatter contracts os0 (as the 1-column weight) against
  PE-transposed permutation tiles into a contiguous [1,512] psum
  row -> one 2KB DMA descriptor per batch.
"""

import numpy as np

import concourse.bacc as bacc
import concourse.mybir as mybir
from concourse import tile
from concourse.bass_utils import run_bass_kernel_spmd

F32 = mybir.dt.float32
F16 = mybir.dt.float16
F8 = mybir.dt.float8e4
BF16 = mybir.dt.bfloat16
I16 = mybir.dt.int16
I32 = mybir.dt.int32

B_FULL = 16
N = 512
NB = N // 128
TK = 50
EPS = 1e-3
N_STEPS = 1  # total chain steps (step 0 contracts w0 = ones; the
             # output's v = 1/(Kt u) acts as an implicit half-step, and
             # rows of P sum to 1 by construction - verified ~400x under
             # the rel-err gate vs the 100-iteration reference)
N_CORES = 8
B_LOC = B_FULL // N_CORES
# beyond this distance exp(-d^2/eps) < 1.2e-7: numerically irrelevant
D_TRIM = float(np.sqrt(16.0 * EPS))
# fp32-exact-zero cutoff, used for the block-0 confinement assert
D_CUT = float(np.sqrt(87.5 * EPS))
RT = float(np.sqrt(1.0 / EPS))  # sqrt(1000)
WARM = 20
WARM2 = 34  # bridges the PE to the first sort matmul (idle downclocks)


def _used_pairs(blocks):
    """(io,jo) pairs actually contracted by the chain + output."""
    needed = [None] * N_STEPS
    needed[N_STEPS - 1] = [0, 1]
    for k in range(N_STEPS - 2, -1, -1):
        req = set()
        for io in needed[k + 1]:
            req.update(blocks[io])
        needed[k] = sorted(req)
    used = set()
    for k in range(N_STEPS):
        for io in needed[k]:
            for jo in blocks[io]:
                used.add((io, jo))
    for jo in blocks[0]:
        used.add((0, jo))  # pv and o50 read block-0 rows
    return needed, used


def _band_structure(scores):
    """Block band + per-(io,jo) trimmed row ranges of the sorted-score
    kernel matrix, unioned over all batches (SPMD: one program runs on
    every core)."""
    t = -np.sort(-scores.astype(np.float64), axis=-1)
    pairs = {}
    for b in range(scores.shape[0]):
        tb = t[b]
        for io in range(NB):
            ta = tb[io * 128 : (io + 1) * 128]
            for jo in range(NB):
                tj = tb[jo * 128 : (jo + 1) * 128]
                dmin = np.abs(ta[:, None] - tj[None, :]).min(axis=1)
                amask = dmin <= D_TRIM
                if not amask.any():
                    continue
                a_lo, a_hi = int(np.argmax(amask)), 128 - int(np.argmax(amask[::-1]))
                # psum matmul outputs must start at partition 0/32/64
                a_lo = min((a_lo // 32) * 32, 64)
                lo0, hi0 = pairs.get((io, jo), (128, 0))
                pairs[(io, jo)] = (min(lo0, a_lo), max(hi0, a_hi))
    blocks = {
        io: sorted(jo for (i, jo) in pairs if i == io) for io in range(NB)
    }
    for io in range(NB):
        assert io in blocks[io]
        assert pairs[(io, io)] == (0, 128)
    return blocks, pairs


def _check_block0_confined(scores):
    """Output (top-50 mass) must vanish for sorted ranks >= 128."""
    t = -np.sort(-scores.astype(np.float64), axis=-1)
    for b in range(scores.shape[0]):
        assert t[b, TK - 1] - t[b, 128] > D_CUT, "top-50 mass leaks past block 0"


def _build(blocks, pairs):
    nc = bacc.Bacc("TRN2", target_bir_lowering=False, debug=False)

    shlr_d = nc.declare_dram_parameter("s_hl_row", [B_LOC, 2, N], F16, isOutput=False)
    shlp_d = nc.declare_dram_parameter(
        "s_hl_parts", [B_LOC, 2 * NB, 128], F16, isOutput=False
    )
    out_d = nc.declare_dram_parameter("out", [B_LOC, N], F32, isOutput=True)

    needed, used = _used_pairs(blocks)
    io_span, tile_lo, tile_hi = {}, {}, {}
    for jo in range(NB):
        ios = sorted(io for (io, j) in used if j == jo)
        if not ios:
            continue
        assert ios == list(range(ios[0], ios[-1] + 1))
        io_span[jo] = ios
        tile_lo[jo] = ios[0] * 128 + pairs[(ios[0], jo)][0]
        tile_hi[jo] = ios[-1] * 128 + pairs[(ios[-1], jo)][1]

    AF = mybir.ActivationFunctionType
    OP = mybir.AluOpType

    with nc.allow_low_precision(reason="fp16 sinkhorn iterates"), \
         tile.TileContext(nc) as tc:
        with tc.tile_pool(name="sb", bufs=1) as sb, \
             tc.tile_pool(name="scr", bufs=2) as scr, \
             tc.tile_pool(name="wp", bufs=2) as wp, \
             tc.tile_pool(name="ps_rep", bufs=1, space="PSUM") as ps_rep, \
             tc.tile_pool(name="ps_tr", bufs=2, space="PSUM") as ps_tr, \
             tc.tile_pool(name="ps_sm", bufs=1, space="PSUM") as ps_sm, \
             tc.tile_pool(name="ps_out", bufs=1, space="PSUM") as ps_out:

            # ---- input DMAs: 4KB per batch, contiguous f16 ----
            shl_row, shl_p = {}, {}
            for b in range(B_LOC):
                shl_row[b] = sb.tile([2, N], F16, name=f"shlr{b}", tag=f"shlr{b}")
                shl_p[b] = sb.tile([2 * NB, 128], F16, name=f"shlp{b}", tag=f"shlp{b}")
            nc.sync.dma_start(shl_row[0][:], shlr_d[0])
            nc.sync.dma_start(shl_p[0][:], shlp_d[0])
            nc.scalar.dma_start(shl_row[1][:], shlr_d[1])
            nc.scalar.dma_start(shl_p[1][:], shlp_d[1])

            # ---- gpsimd: iotas ----
            iota_i = scr.tile([128, N], I16, name="iota_i", tag="iota_i")
            nc.gpsimd.iota(iota_i[:], pattern=[[1, N]], base=0,
                           channel_multiplier=0)
            iotac_i = scr.tile([128, 1], I32, name="iotac_i", tag="iotac_i")
            nc.gpsimd.iota(iotac_i[:], pattern=[[1, 1]], base=0,
                           channel_multiplier=1)

            # ---- DVE: consts, casts ----
            dummy16 = sb.tile([1, 128], F16, name="dummy16", tag="dummy16")
            nc.vector.memset(dummy16[:], 1.0)
            ones21 = sb.tile([2, 1], F16, name="ones21", tag="ones21")
            nc.vector.memset(ones21[:], 1.0)
            ones2r = sb.tile([2, 128], F16, name="ones2r", tag="ones2r")
            nc.vector.memset(ones2r[:], 1.0)
            w0ones = sb.tile([128, 1], F16, name="w0ones", tag="w0ones")
            nc.vector.memset(w0ones[:], 1.0)
            iota_rep = sb.tile([128, N], F16, name="iota_rep", tag="iota_rep")
            nc.vector.tensor_copy(iota_rep[:], iota_i[:])
            iota_col = sb.tile([128, 1], F32, name="iota_col", tag="iota_col")
            nc.vector.tensor_copy(iota_col[:], iotac_i[:])
            identity = sb.tile([128, 128], F16, name="identity", tag="identity")
            nc.vector.tensor_scalar(
                out=identity[:], in0=iota_rep[:, 0:128], scalar1=iota_col[:],
                scalar2=None, op0=OP.is_equal,
            )
            identity8 = sb.tile([128, 128], F8, name="identity8", tag="identity8")
            nc.vector.tensor_copy(identity8[:], identity[:])
            mask50 = sb.tile([128, 1], F16, name="mask50", tag="mask50")
            nc.vector.tensor_scalar(
                out=mask50[:], in0=iota_col[:], scalar1=float(TK),
                scalar2=None, op0=OP.is_lt,
            )

            # ---- ACT: force the erf_derivative table set to load now ----
            derf_warm = sb.tile([1, 128], F16, name="derf_warm", tag="derf_warm")
            nc.scalar.activation(derf_warm[:], dummy16[:], AF.Derivative_Erf)

            # ---- PE: warm-up; s_rep broadcasts; s_hl transposes ----
            warm_ps = ps_out.tile([128, 128], F32, name="warm", tag="or0")
            for _ in range(WARM):
                nc.tensor.matmul(
                    warm_ps[:], dummy16[:], dummy16[:], start=True, stop=True
                )
            s_rep_ps, shlt_ps = {}, {}
            for b in range(B_LOC):
                s_rep_ps[b] = ps_rep.tile([128, N], F32, name=f"srep{b}", tag=f"rep{b}")
                nc.tensor.matmul(
                    s_rep_ps[b][:], ones2r[:], shl_row[b][:], start=True, stop=True
                )
                # s_hlT[p, m] = shl_p[m, p]: the [128, 2NB] sort weights
                shlt_ps[b] = ps_tr.tile([128, 2 * NB], F16, name=f"shlt{b}", tag="tp")
                nc.tensor.transpose(
                    shlt_ps[b][:], shl_p[b][:], identity[0 : 2 * NB, 0 : 2 * NB]
                )
            for _ in range(WARM2):
                nc.tensor.matmul(
                    warm_ps[:], dummy16[:], dummy16[:], start=True, stop=True
                )

            s_hl, spa = {}, {}
            rankv, asum, pm = {}, {}, {}
            t_row_ps, t2h, t_rep_ps, tcol_ps, ntcol = {}, {}, {}, {}, {}
            kw, w16 = {}, {}
            tpose_ps, pmT = {}, {}
            u50, v0, os0 = {}, {}, {}

            def emit_shl(b):
                # spa = h+l is the rank scalar (must match s_rep's h+l sum
                # exactly; see module doc) - read straight from psum so the
                # rank ops don't wait for the sbuf weight copy
                s_hl[b] = sb.tile([128, 2 * NB], F16, name=f"s_hl{b}", tag=f"s_hl{b}")
                nc.vector.tensor_copy(s_hl[b][:], shlt_ps[b][:])
                spa[b] = sb.tile([128, NB], F32, name=f"spa{b}", tag=f"spa{b}")
                nc.vector.tensor_tensor(
                    out=spa[b][:], in0=s_hl[b][:, 0 : 2 * NB : 2],
                    in1=shlt_ps[b][:, 1 : 2 * NB : 2], op=OP.add,
                )

            def emit_ranks_dve(b):
                rankv[b] = sb.tile([128, NB], F32, name=f"rank{b}", tag=f"rank{b}")
                for c in range(NB):
                    junk = scr.tile([128, N], BF16, name=f"cmp{b}", tag=f"cmp{b}")
                    nc.vector.tensor_scalar(
                        out=junk[:],
                        in0=s_rep_ps[b][:],
                        scalar1=spa[b][:, c : c + 1],
                        scalar2=0.0,
                        op0=OP.is_gt,
                        op1=OP.add,
                        accum_out=rankv[b][:, c : c + 1],
                    )

            def emit_ranks_act(b):
                asum[b] = sb.tile([128, NB], F32, name=f"asum{b}", tag=f"asum{b}")
                for c in range(NB):
                    junk = scr.tile([128, N], BF16, name=f"sgn{b}", tag=f"sgn{b}")
                    nc.scalar.activation(
                        junk[:], s_rep_ps[b][:], AF.Sign,
                        bias=spa[b][:, c : c + 1], scale=-1.0,
                        accum_out=asum[b][:, c : c + 1],
                    )

            def emit_rank_transform(b):
                rankv[b] = sb.tile([128, NB], F32, name=f"rank{b}", tag=f"rank{b}")
                nc.vector.tensor_scalar(
                    out=rankv[b][:], in0=asum[b][:], scalar1=-0.5, scalar2=255.5,
                    op0=OP.mult, op1=OP.add,
                )

            def emit_pm(b):
                for c in range(NB):
                    pmt = sb.tile([128, N], F16, name=f"pm{b}_{c}", tag=f"pm{b}_{c}")
                    nc.vector.tensor_scalar(
                        out=pmt[:],
                        in0=iota_rep[:],
                        scalar1=rankv[b][:, c : c + 1],
                        scalar2=None,
                        op0=OP.is_equal,
                    )
                    pm[(b, c)] = pmt

            def emit_sort_mms(b):
                t_row_ps[b] = ps_sm.tile([2, N], F32, name=f"trps{b}", tag=f"sm{b}")
                for c in range(NB):
                    nc.tensor.matmul(
                        t_row_ps[b][:],
                        s_hl[b][:, 2 * c : 2 * c + 2],
                        pm[(b, c)][:],
                        start=(c == 0),
                        stop=(c == NB - 1),
                    )

            def emit_t2h(b):
                # exact f16 re-split of the sorted h/l rows
                t2h[b] = sb.tile([2, N], F16, name=f"t2h{b}", tag=f"t2h{b}")
                if b == 0:
                    nc.vector.tensor_copy(t2h[b][:], t_row_ps[b][:])
                else:
                    nc.scalar.copy(t2h[b][:], t_row_ps[b][:])

            def emit_tcol_trep_mms(b):
                tcol_ps[b] = ps_sm.tile([128, NB], F32, name=f"tcps{b}", tag=f"sm{b}")
                for c in range(NB):
                    nc.tensor.matmul(
                        tcol_ps[b][:, c : c + 1],
                        t2h[b][:, c * 128 : (c + 1) * 128],
                        ones21[:],
                        start=True,
                        stop=True,
                    )
                t_rep_ps[b] = ps_rep.tile([128, N], F32, name=f"trep{b}", tag=f"rep{b}")
                nc.tensor.matmul(
                    t_rep_ps[b][:], ones2r[:], t2h[b][:], start=True, stop=True
                )

            def emit_kt(b):
                for jo in sorted(io_span):
                    lo, hi = tile_lo[jo], tile_hi[jo]
                    dt = scr.tile([128, hi - lo], F16, name=f"d{b}", tag=f"d{b}")
                    nc.vector.tensor_scalar(
                        out=dt[:],
                        in0=t_rep_ps[b][:, lo:hi],
                        scalar1=tcol_ps[b][:, jo : jo + 1],
                        scalar2=RT,
                        op0=OP.subtract,
                        op1=OP.mult,
                    )
                    kt = sb.tile([128, hi - lo], F16, name=f"kt{b}_{jo}", tag=f"kt{b}_{jo}")
                    nc.scalar.activation(kt[:], dt[:], AF.Derivative_Erf)
                    kw[(b, jo)] = kt

            def _mm_io(b, pw, io, jo, rhs_col, start, stop):
                a_lo, a_hi = pairs[(io, jo)]
                c_lo = io * 128 + a_lo - tile_lo[jo]
                c_hi = io * 128 + a_hi - tile_lo[jo]
                nc.tensor.matmul(
                    pw[a_lo:a_hi, io : io + 1],
                    kw[(b, jo)][:, c_lo:c_hi],
                    rhs_col,
                    start=start,
                    stop=stop,
                )

            def emit_step(b, k):
                ios = needed[k]
                ncols = ios[-1] + 1
                pw = ps_sm.tile([128, NB], F32, name=f"pw{b}", tag=f"sm{b}")
                for io in ios:
                    jos = [io] + [j for j in blocks[io] if j != io]
                    for ji, jo in enumerate(jos):
                        rhs = w0ones[:] if k == 0 else w16[b][:, jo : jo + 1]
                        _mm_io(b, pw, io, jo, rhs, ji == 0, ji == len(jos) - 1)
                wn = wp.tile([128, NB], F16, name=f"w{b}", tag=f"w{b}")
                nc.vector.reciprocal(wn[:, 0:ncols], pw[:, 0:ncols])
                w16[b] = wn

            def emit_pm_transposes(b):
                for c in range(NB):
                    tp = ps_tr.tile([128, 128], F16, name=f"tp{b}_{c}", tag="tp")
                    nc.tensor.transpose(tp[:], pm[(b, c)][:, 0:128], identity[:])
                    tpose_ps[(b, c)] = tp

            def emit_pmT_copies(b):
                for c in range(NB):
                    pt = sb.tile([128, 128], F16, name=f"pmT{b}_{c}", tag=f"pmT{b}_{c}")
                    nc.scalar.copy(pt[:], tpose_ps[(b, c)][:])
                    pmT[(b, c)] = pt

            def emit_u50(b):
                u50[b] = sb.tile([128, 1], F16, name=f"u50{b}", tag=f"u50{b}")
                nc.vector.tensor_tensor(
                    out=u50[b][:], in0=w16[b][:, 0:1], in1=mask50[:], op=OP.mult
                )

            def emit_pv(b):
                pv = ps_sm.tile([128, NB], F32, name=f"pv{b}", tag=f"sm{b}")
                jos0 = [0] + [j for j in blocks[0] if j != 0]
                for ji, jo in enumerate(jos0):
                    _mm_io(b, pv, 0, jo, w16[b][:, jo : jo + 1],
                           ji == 0, ji == len(jos0) - 1)
                nc.tensor.matmul(
                    pv[:, 1:2],
                    kw[(b, 0)][:, 0 - tile_lo[0] : 128 - tile_lo[0]],
                    u50[b][:],
                    start=True,
                    stop=True,
                )
                return pv

            def emit_os0(b, pv):
                v0[b] = sb.tile([128, 1], F32, name=f"v0{b}", tag=f"v0{b}")
                nc.vector.reciprocal(v0[b][:], pv[:, 0:1])
                os0[b] = sb.tile([128, 1], F16, name=f"os0{b}", tag=f"os0{b}")
                nc.vector.tensor_tensor(
                    out=os0[b][:], in0=v0[b][:], in1=pv[:, 1:2], op=OP.mult
                )

            def emit_scatter(b):
                orp = ps_out.tile([1, N], F32, name=f"or{b}", tag=f"or{b}")
                for c in range(NB):
                    nc.tensor.matmul(
                        orp[0:1, c * 128 : (c + 1) * 128],
                        os0[b][:],
                        pmT[(b, c)][:],
                        start=True,
                        stop=True,
                    )
                out_row = sb.tile([1, N], F32, name=f"orow{b}", tag=f"orow{b}")
                nc.vector.tensor_copy(out_row[:], orp[:])
                nc.sync.dma_start(
                    out_d[b].rearrange("(o n) -> o n", o=1), out_row[:]
                )

            # ---- emission schedule ----
            emit_shl(0)
            emit_shl(1)
            emit_ranks_act(1)   # ACT Sign b1
            emit_ranks_dve(0)   # DVE counts b0 concurrently
            emit_pm(0)
            emit_rank_transform(1)
            emit_pm(1)
            emit_sort_mms(0)
            emit_t2h(0)
            emit_pm_transposes(0)
            emit_pmT_copies(0)
            emit_tcol_trep_mms(0)
            emit_sort_mms(1)
            emit_t2h(1)
            emit_pm_transposes(1)
            emit_pmT_copies(1)
            emit_tcol_trep_mms(1)
            emit_kt(0)
            emit_kt(1)
            for k in range(N_STEPS):
                emit_step(0, k)
            emit_u50(0)
            pv0 = emit_pv(0)
            emit_os0(0, pv0)
            for k in range(N_STEPS):
                emit_step(1, k)
            emit_u50(1)
            pv1 = emit_pv(1)
            emit_os0(1, pv1)
            emit_scatter(1)
            emit_scatter(0)

    nc.compile()
    return nc


def kernel(scores):
    scores = np.ascontiguousarray(np.asarray(scores, dtype=np.float32))
    assert scores.shape == (B_FULL, N)
    h = scores.astype(np.float16)
    l = (scores - h.astype(np.float32)).astype(np.float16)
    approx = h.astype(np.float32) + l.astype(np.float32)
    for b in range(B_FULL):
        # the comparison-count sort assumes distinct scores per batch,
        # including after the exact-fp16-pair approximation (~2^-22 rel)
        assert np.unique(scores[b]).size == N, "tied scores unsupported"
        assert np.unique(approx[b]).size == N, "h+l approximation ties"
    blocks, pairs = _band_structure(scores)
    _check_block0_confined(scores)
    nc = _build(blocks, pairs)

    # h/l chunk columns in [2*NB, 128] layout: row 2c = h of chunk c
    hl_parts = np.empty((B_FULL, 2 * NB, 128), np.float16)
    for c in range(NB):
        hl_parts[:, 2 * c] = h[:, c * 128 : (c + 1) * 128]
        hl_parts[:, 2 * c + 1] = l[:, c * 128 : (c + 1) * 128]

    in_maps = []
    for cr in range(N_CORES):
        sl = slice(cr * B_LOC, (cr + 1) * B_LOC)
        in_maps.append({
            "s_hl_row": np.ascontiguousarray(np.stack([h[sl], l[sl]], axis=1)),
            "s_hl_parts": np.ascontiguousarray(hl_parts[sl]),
        })
    res = run_bass_kernel_spmd(nc, in_maps, core_ids=list(range(N_CORES)))
    return np.concatenate(
        [res.results[cr]["out"] for cr in range(N_CORES)], axis=0
    ).astype(np.float32)
